# revision 9
# baseline (speedup 1.0000x reference)
"""Chamfer loss (nn_ChamferLoss) Trainium2 Bass kernel.

Problem: x, y: [B=4, D=3, N=M=8192] fp32. Output: scalar
    dist = mean_b mean_n min_m d2[b,n,m] + mean_b mean_m min_n d2[b,n,m]
    d2 = |x_n|^2 + |y_m|^2 - 2 x_n.y_m

Strategy
--------
* Host: augment points to 5 dims so the PE emits squared distances directly:
    xa = [-2*x0, -2*x1, -2*x2, |x|^2, 1]   (K=5 rows)
    ya = [ y0,    y1,    y2,   1, |y|^2]
  => xa . ya = |x|^2 + |y|^2 - 2 x.y = d2
* Sharding: 8 cores = 4 batches x 2 halves of N.  Each core computes its
  [4096, 8192] distance block with K=5 f32r matmuls (1 cyc/row), converts
  PSUM->fp16 SBUF on the Scalar engine, and does all mins on the Vector
  engine in 16-bit 2x/4x perf modes:
    - row mins  : tensor_scalar(min) with accum_out (fused free-axis min)
    - col mins  : tensor_tensor(min) accumulation over row tiles
  Column partial mins are partition-reduced via PE transpose + reduce.
* Host: combine per-core row-min sums and column partial mins, final means.
"""

import os
import numpy as np
from contextlib import ExitStack

import concourse.bass as bass
import concourse.mybir as mybir
import concourse.tile as tile
from concourse import bacc
from concourse.bass_utils import run_bass_kernel_spmd
from concourse.masks import make_identity

B, D, N, M = 4, 3, 8192, 8192
NCORES = 8
NHALF = N // 2            # rows per core
P = 128                   # partitions
NT = NHALF // P           # 32 row tiles per core
MT = 512                  # matmul moving free size (one PSUM bank fp32)
CHUNK = 2048              # ACT convert / DVE op width (4 matmul tiles)
NCHUNK = M // CHUNK       # 4 chunks per row tile
KA = 7                    # augmented contraction dim (hi/lo norm splits)
NBLK = M // P             # 64 column blocks for the transpose tail

F32 = mybir.dt.float32
F32R = mybir.dt.float32r
F16 = mybir.dt.float16

BIG = 3.0e38              # identity element for min in fp32

_cached_nc = None
last_results = None       # test.py reads exec_time_ns etc. from here


def _build():
    """Build and compile the per-core SPMD program (same on all 8 cores)."""
    global _cached_nc
    if _cached_nc is not None:
        return _cached_nc

    nc = bacc.Bacc("TRN2", target_bir_lowering=False, debug=False,
                   num_devices=NCORES)

    xt = nc.dram_tensor("xt", [KA, NHALF], F32R, kind="ExternalInput").ap()
    yt = nc.dram_tensor("yt", [KA, M], F32R, kind="ExternalInput").ap()
    rowres_d = nc.dram_tensor("rowres", [P, NT], F32, kind="ExternalOutput").ap()
    colres_d = nc.dram_tensor("colres", [P, NBLK], F32, kind="ExternalOutput").ap()

    with tile.TileContext(nc) as tc, ExitStack() as ctx:
        consts = ctx.enter_context(tc.tile_pool(name="consts", bufs=1))
        accs = ctx.enter_context(tc.tile_pool(name="accs", bufs=1))
        conv_pool = ctx.enter_context(tc.tile_pool(name="conv", bufs=6))
        scr_pool = ctx.enter_context(tc.tile_pool(name="scr", bufs=2))
        rmin_pool = ctx.enter_context(tc.tile_pool(name="rmin", bufs=2))

        xs = consts.tile([KA, NHALF], F32R)
        nc.sync.dma_start(out=xs[:], in_=xt)
        ys = consts.tile([KA, M], F32R)
        nc.sync.dma_start(out=ys[:], in_=yt)

        cacc = accs.tile([P, M], F16)         # column partial mins
        rowres = accs.tile([P, NT], F32)      # per-row mins (one col per tile)
        colres = accs.tile([P, NBLK], F32)

        with tc.tile_pool(name="psum", bufs=2, space="PSUM") as psum_pool:
            for t in range(NT):
                lhsT = xs[:, t * P:(t + 1) * P]     # [KA, 128] f32r
                rmin4 = rmin_pool.tile([P, NCHUNK], F32)
                for g in range(NCHUNK):
                    ps = psum_pool.tile([P, CHUNK], F32, tag="ps")
                    for j in range(CHUNK // MT):
                        m0 = g * CHUNK + j * MT
                        nc.tensor.matmul(
                            ps[:, j * MT:(j + 1) * MT],
                            lhsT,
                            ys[:, m0:m0 + MT],
                            start=True, stop=True,
                        )
                    conv = conv_pool.tile([P, CHUNK], F16, tag="conv")
                    nc.scalar.copy(conv[:], ps[:])          # ACT: fp32->fp16
                    # fused row-min of this chunk (DVE 4x mode path)
                    scr = scr_pool.tile([P, CHUNK], F16, tag="scr")
                    nc.vector.tensor_scalar(
                        scr[:], conv[:], BIG, None,
                        op0=mybir.AluOpType.min, op1=mybir.AluOpType.min,
                        accum_out=rmin4[:, g:g + 1],
                    )
                    # column-min accumulate (DVE 2x tensor_tensor)
                    csl = cacc[:, g * CHUNK:(g + 1) * CHUNK]
                    if t == 0:
                        nc.vector.tensor_copy(csl, conv[:])
                    else:
                        nc.vector.tensor_tensor(csl, csl, conv[:],
                                                op=mybir.AluOpType.min)
                nc.vector.tensor_reduce(rowres[:, t:t + 1], rmin4[:],
                                        axis=mybir.AxisListType.X,
                                        op=mybir.AluOpType.min)

        # Tail: column partial mins across partitions via PE transpose.
        ident = consts.tile([P, P], F16)
        make_identity(nc, ident[:])
        with tc.tile_pool(name="tpsum", bufs=4, space="PSUM") as tail_psum:
            for blk in range(NBLK):
                tp = tail_psum.tile([P, P], F16, tag="tp")
                nc.tensor.transpose(tp[:], cacc[:, blk * P:(blk + 1) * P],
                                    ident[:])
                nc.vector.tensor_reduce(colres[:, blk:blk + 1], tp[:],
                                        axis=mybir.AxisListType.X,
                                        op=mybir.AluOpType.min)

        nc.sync.dma_start(out=rowres_d, in_=rowres[:])
        nc.sync.dma_start(out=colres_d, in_=colres[:])

    nc.compile()
    _cached_nc = nc
    return nc


def _f32r_round(a):
    """Round fp32 to the PE's f32r format: 1s + 8e + 11m (top 20 bits), RNE."""
    u = np.ascontiguousarray(a, np.float32).view(np.uint32).astype(np.uint64)
    lsb = (u >> 12) & 1
    u = ((u + 0x7FF + lsb) >> 12) << 12
    return (u & 0xFFFFFFFF).astype(np.uint32).view(np.float32)


def _augment(x, y):
    """Host-side augmentation. x,y: [B, 3, N] fp32 -> xa,ya: [B, 7, *] f32r.

    Points are pre-rounded to f32r so the PE computes the exact squared
    distance between the *rounded* points: |xr|^2 is computed from xr and
    carried as f32r hi + residual lo rows (both exactly representable up
    to ~1e-7), preserving the |xr-yr|^2 cancellation structure.
    """
    xr = _f32r_round(x)
    yr = _f32r_round(y)
    ones = np.ones((x.shape[0], 1, x.shape[2]), np.float32)

    def hilo(sq):
        hi = _f32r_round(sq)
        lo = _f32r_round(sq - hi)
        return hi[:, None, :], lo[:, None, :]

    xsq_hi, xsq_lo = hilo(np.sum(xr * xr, axis=1, dtype=np.float32))
    ysq_hi, ysq_lo = hilo(np.sum(yr * yr, axis=1, dtype=np.float32))
    xa = np.concatenate([-2.0 * xr, xsq_hi, xsq_lo, ones, ones],
                        axis=1).astype(np.float32)
    ya = np.concatenate([yr, ones, ones, ysq_hi, ysq_lo],
                        axis=1).astype(np.float32)
    return xa, ya


def kernel(x, y):
    global last_results
    x = np.ascontiguousarray(np.asarray(x, dtype=np.float32))
    y = np.ascontiguousarray(np.asarray(y, dtype=np.float32))
    assert x.shape == (B, D, N) and y.shape == (B, D, M)

    xa, ya = _augment(x, y)

    in_maps = []
    for c in range(NCORES):
        b, h = divmod(c, 2)
        in_maps.append({
            "xt": np.ascontiguousarray(xa[b, :, h * NHALF:(h + 1) * NHALF]),
            "yt": np.ascontiguousarray(ya[b]),
        })

    nc = _build()
    res = run_bass_kernel_spmd(nc, in_maps, list(range(NCORES)))
    last_results = res

    cham_x = 0.0
    cham_y = 0.0
    for b in range(B):
        r0 = res.results[2 * b]
        r1 = res.results[2 * b + 1]
        row_sum = (r0["rowres"].astype(np.float64).sum()
                   + r1["rowres"].astype(np.float64).sum())
        colmin = np.minimum(r0["colres"], r1["colres"])
        col_sum = colmin.astype(np.float64).sum()
        cham_x += row_sum / N
        cham_y += col_sum / M
    dist = cham_x / B + cham_y / B
    return np.float32(dist)
